# revision 1
# baseline (speedup 1.0000x reference)
"""ASTGCN block Trainium2 kernel.

Strategy: 8 cores; core c handles batch b = c//2, time-half h = c%2 (8 output
timesteps each, data-parallel over B and T). Attention (temporal Et, spatial
S) is per-b and replicated on the 2 cores sharing a b. The sparse graph
propagation is reformulated as dense (N,N) matmuls: the edge-scatter of the
symmetric norm is accumulated host-side into a dense W (the +I/-I self-loop
terms cancel), so  prop1(h) = (W*S) @ h  and  prop2(h) = W @ h.

Per-core time axis is PERMUTED so the program is identical SPMD: slot t' maps
to global t via tmap (identity for h=0, rotated by 6 for h=1); all
t-dependent weights (be, Ve, Ws1, UW) are permuted host-side to match.

All matmul operands are bf16 (fp32 PSUM accumulate); LayerNorm stats run in
fp32. Cheb/conv run on (v,f)x n "pair" tiles (2 timesteps per 128-partition
tile); outputs are PE-transposed back to n-partition as (n, slot, f).
"""

import numpy as np

B, N, F, T = 4, 512, 64, 16
P = 128
CH = N // P            # 4 n-chunks
NSLOT = 10             # cheb window timesteps per core (5 pairs)
LN_EPS = 1e-5

_CACHE = {}


def _build_program(stage=4):
    import sys
    if '/opt/trn_rl_repo' not in sys.path:
        sys.path.insert(0, '/opt/trn_rl_repo')
    from contextlib import ExitStack
    import concourse.bass as bass
    import concourse.tile as tile
    from concourse import bacc, mybir

    dt = mybir.dt
    AL = mybir.AluOpType
    AF = mybir.ActivationFunctionType
    AX = mybir.AxisListType
    f32 = dt.float32
    bf16 = dt.bfloat16

    nc = bacc.Bacc("TRN2", target_bir_lowering=False, debug=False, num_devices=1)

    def din(name, shape, d=bf16):
        return nc.dram_tensor(name, list(shape), d, kind="ExternalInput").ap()

    XnD   = din("Xn", (N, F * T))
    XwD   = din("Xw", (8, P, N))
    UWD   = din("UW", (8, P, 48))
    bsD   = din("bs", (N, N))
    VsTD  = din("VsT", (N, N))
    WTD   = din("WT", (N, N))
    WpkD  = din("Wpk", (7, P, P))
    PbD   = din("Pb", (P, 1061))
    PfD   = din("Pf", (P, 274), f32)
    ZoutD = nc.dram_tensor("Zout", [N, NSLOT * F], f32, kind="ExternalOutput").ap()

    with tile.TileContext(nc) as tc, ExitStack() as ctx:
        sg = ctx.enter_context(tc.tile_pool(name="sg", bufs=1))

        # ---------------- load everything (few big DMAs) ----------------
        XnAll = sg.tile([P, CH, F * T], bf16, tag="xnall")
        nc.sync.dma_start(out=XnAll[:], in_=XnD.rearrange("(k p) t -> p k t", k=CH))
        XwAll = sg.tile([P, 8, N], bf16, tag="xwall")
        nc.sync.dma_start(out=XwAll[:], in_=XwD.rearrange("s p n -> p s n"))
        UWAll = sg.tile([P, 8, 48], bf16, tag="uwall")
        nc.sync.dma_start(out=UWAll[:], in_=UWD.rearrange("s p n -> p s n"))
        bsAll = sg.tile([P, CH, N], bf16, tag="bsall")
        nc.sync.dma_start(out=bsAll[:], in_=bsD.rearrange("(k p) n -> p k n", k=CH))
        VsTAll = sg.tile([P, CH, N], bf16, tag="vstall")
        nc.sync.dma_start(out=VsTAll[:], in_=VsTD.rearrange("(k p) n -> p k n", k=CH))
        WTAll = sg.tile([P, CH, N], bf16, tag="wtall")
        nc.sync.dma_start(out=WTAll[:], in_=WTD.rearrange("(k p) n -> p k n", k=CH))
        Wpk = sg.tile([P, 7, P], bf16, tag="wpk")
        nc.sync.dma_start(out=Wpk[:], in_=WpkD.rearrange("w p c -> p w c"))
        Pb = sg.tile([P, 1061], bf16, tag="pb")
        nc.sync.dma_start(out=Pb[:], in_=PbD)
        Pf = sg.tile([P, 274], f32, tag="pf")
        nc.sync.dma_start(out=Pf[:], in_=PfD)

        Xn = [XnAll[:, k, :] for k in range(CH)]
        Xw = [XwAll[:, s, :] for s in range(8)]
        UW = [UWAll[:, s, :] for s in range(8)]
        bs = [bsAll[:, k, :] for k in range(CH)]
        VsT = [VsTAll[:, k, :] for k in range(CH)]
        WT = [WTAll[:, k, :] for k in range(CH)]
        WcP = [Wpk[:, k, :] for k in range(3)]
        Lprev, Lmid, Lnext, WrP = (Wpk[:, 3, :], Wpk[:, 4, :], Wpk[:, 5, :],
                                   Wpk[:, 6, :])
        # packed bf16: U1r[0:4] Ws2d[4:20] VeT(r16)[20:36] Ws1(r16)[36:37]
        #              ones1(r1)[37:165] I128b[165:293] U2(r64)[293:805]
        U1r = Pb[:, 0:4]
        Ws2d = Pb[:, 4:20]
        VeT = Pb[0:16, 20:36]
        Ws1 = Pb[0:16, 36:37]
        ones1 = Pb[0:1, 37:165]
        I128b = Pb[:, 165:293]
        U2 = Pb[0:64, 293:805]
        I16r = Pb[0:1, 805:1061]     # I16 rows flattened: e_t = [0:1, 16t:16t+16]
        # packed f32: gamB[0:64] betB[64:128] bch[128:129] btr[129:130]
        #             I128f[130:258] beP(r16)[258:274]
        gamB = Pf[:, 0:64]
        betB = Pf[:, 64:128]
        bch = Pf[:, 128:129]
        btr = Pf[:, 129:130]
        I128f = Pf[:, 130:258]
        beP = Pf[0:16, 258:274]

        zerot = sg.tile([P, N], bf16, tag="zerot")
        nc.vector.memset(zerot[:], 0.0)
        epsP = sg.tile([P, 1], f32, tag="epsP")
        nc.vector.memset(epsP[:], LN_EPS)

        # persistent sbuf intermediates
        G = [sg.tile([P, N], bf16, tag=f"g{k}", name=f"g{k}") for k in range(CH)]
        Ex = [sg.tile([P, N], bf16, tag=f"ex{k}", name=f"ex{k}") for k in range(CH)]
        A1T = [sg.tile([P, N], bf16, tag=f"a1t{k}", name=f"a1t{k}") for k in range(CH)]
        dSv = [sg.tile([P, 1], f32, tag=f"dsv{k}", name=f"dsv{k}") for k in range(CH)]
        Tx0n = [sg.tile([P, F * T], bf16, tag=f"tx0n{k}", name=f"tx0n{k}") for k in range(CH)]
        dSB = sg.tile([P, N], bf16, tag="dsb")
        Zbig = [sg.tile([P, NSLOT * F], bf16, tag=f"zbig{k}", name=f"zbig{k}") for k in range(CH)]
        lnout = [sg.tile([P, NSLOT * F], f32, tag=f"lnout{k}", name=f"lnout{k}") for k in range(CH)]

        # =====================================================
        # Attention phase
        # =====================================================
        with tc.tile_pool(name="pss", bufs=3, space="PSUM") as pss, \
             tc.tile_pool(name="psP", bufs=2, space="PSUM") as psP, \
             tc.tile_pool(name="psM", bufs=3, space="PSUM") as psM:

            # ---- lhs0[f,t'] = sum_n U1[n] X[n,f,t']  -> (1,1024) -> (64,16)
            L0a = pss.tile([1, 512], f32, tag="pssT", name="l0a")
            L0b = pss.tile([1, 512], f32, tag="pssT", name="l0b")
            for k in range(CH):
                nc.tensor.matmul(L0a[:], U1r[:, k:k + 1], Xn[k][:, 0:512],
                                 start=(k == 0), stop=(k == CH - 1))
            for k in range(CH):
                nc.tensor.matmul(L0b[:], U1r[:, k:k + 1], Xn[k][:, 512:1024],
                                 start=(k == 0), stop=(k == CH - 1))
            lhs0row = sg.tile([1, F * T], bf16, tag="lhs0row")
            nc.vector.tensor_copy(lhs0row[:, 0:512], L0a[:])
            nc.vector.tensor_copy(lhs0row[:, 512:1024], L0b[:])
            # L0 free order is (t, f): reshape to (64,16) via 16 rank-1 matmuls
            # against identity rows (outer products accumulate per column)
            l0Fp = pss.tile([F, T], f32, tag="pssT", name="l0fp")
            for t in range(T):
                nc.tensor.matmul(l0Fp[:], lhs0row[0:1, 64 * t:64 * t + 64],
                                 I16r[0:1, 16 * t:16 * t + 16],
                                 start=(t == 0), stop=(t == T - 1))
            lhs0F = sg.tile([F, T], bf16, tag="lhs0f")
            nc.vector.tensor_copy(lhs0F[:], l0Fp[:])

            # ---- lhs2T chunks (n,16) = U2[:,chunk].T @ lhs0F
            lhs2T = []
            for k in range(CH):
                pt = pss.tile([P, T], f32, tag="pssT", name="l2t")
                nc.tensor.matmul(pt[:], U2[:, k * P:(k + 1) * P], lhs0F[:],
                                 start=True, stop=True)
                st = sg.tile([P, T], bf16, tag=f"l2ts{k}", name=f"l2ts{k}")
                nc.vector.tensor_copy(st[:], pt[:])
                lhs2T.append(st)

            # ---- R48: rows 0:16 rhs3T (Ws3), rows 32:48 rhs_tT (U3)
            R48p = pss.tile([48, N], f32, tag="pssT", name="r48")
            for s in range(8):
                # UW rows 0:64 (v=0) and 64:128 (v=1) hit disjoint columns, so
                # one K=128 matmul covers both timesteps of the pair
                nc.tensor.matmul(R48p[:], UW[s][:, :], Xw[s][:, :],
                                 start=(s == 0), stop=(s == 7))
            R48 = sg.tile([48, N], bf16, tag="r48s")
            nc.scalar.copy(R48[:], R48p[:])

            # ---- rhs_tn chunks: transpose R48[32:48]
            rhs_tn = []
            for k in range(CH):
                pt = pss.tile([P, T], bf16, tag="pssT", name="rtn")
                nc.tensor.transpose(pt[:], R48[32:48, k * P:(k + 1) * P],
                                    I128b[32:48, 32:48])
                st = sg.tile([P, T], bf16, tag=f"rtns{k}", name=f"rtns{k}")
                nc.vector.tensor_copy(st[:], pt[:])
                rhs_tn.append(st)

            # ---- P0 (16,16) = lhs_t @ rhs_t ; sigmoid(P0+be)
            P0p = pss.tile([T, T], f32, tag="pssT", name="p0")
            for k in range(CH):
                nc.tensor.matmul(P0p[:], lhs2T[k][:], rhs_tn[k][:],
                                 start=(k == 0), stop=(k == CH - 1))
            sig = sg.tile([T, T], bf16, tag="sig")
            nc.vector.tensor_tensor(sig[:], P0p[:], beP[:], op=AL.add)
            nc.scalar.activation(sig[:], sig[:], AF.Sigmoid)

            # ---- E1^T = sig^T @ Ve^T directly ; softmax over free dim
            E1Tp = pss.tile([T, T], f32, tag="pssT", name="e1t")
            nc.tensor.matmul(E1Tp[:], sig[:], VeT[:], start=True, stop=True)
            E1Ts = sg.tile([T, T], bf16, tag="e1ts")
            nc.vector.tensor_copy(E1Ts[:], E1Tp[:])
            # values are O(1e-1): skip the max-subtraction for softmax
            sume = sg.tile([T, 1], f32, tag="sume")
            EtT = sg.tile([T, T], bf16, tag="ett")
            nc.scalar.activation(EtT[:], E1Ts[:], AF.Exp,
                                 scale=1.0, accum_out=sume[:, 0:1])
            rse = sg.tile([T, 1], f32, tag="rse")
            nc.vector.reciprocal(rse[:], sume[:])
            nc.vector.tensor_scalar(EtT[:], EtT[:], rse[:, 0:1], None, op0=AL.mult)
            Etp = pss.tile([T, T], bf16, tag="pssT", name="etp")
            nc.tensor.transpose(Etp[:], EtT[:], I128b[0:16, 0:16])
            Et = sg.tile([T, T], bf16, tag="et")
            nc.vector.tensor_copy(Et[:], Etp[:])

            # ---- w1e row (1,16) = Ws1.T @ EtT ; broadcast to (128,16)
            w1p = pss.tile([1, T], f32, tag="pssT", name="w1p")
            nc.tensor.matmul(w1p[:], Ws1[:], EtT[:], start=True, stop=True)
            w1row = sg.tile([1, T], bf16, tag="w1row")
            nc.scalar.copy(w1row[:], w1p[:])
            w1Bp = pss.tile([P, T], f32, tag="pssT", name="w1bp")
            nc.tensor.matmul(w1Bp[:], ones1[:], w1row[:], start=True, stop=True)
            w1B = sg.tile([P, T], bf16, tag="w1b")
            nc.vector.tensor_copy(w1B[:], w1Bp[:])

            # ---- w1Bpair[p=(v,f), s] = w1e[2s+v]
            w1Bp2 = sg.tile([P, 8], bf16, tag="w1bp2")
            nc.vector.tensor_copy(w1Bp2[0:64, :], w1B[0:64, 0:T:2])
            nc.vector.tensor_copy(w1Bp2[64:128, :], w1B[64:128, 1:T:2])
            # ---- Ws2wP[p=(v,f), s, t] = Ws2d[p,t] * w1e[2s+v]   (128, 8, 16)
            Ws2w = sg.tile([P, 8, T], bf16, tag="ws2w")
            nc.vector.tensor_tensor(
                Ws2w[:],
                Ws2d[:].unsqueeze(1).broadcast_to((P, 8, T)),
                w1Bp2[:].unsqueeze(2).broadcast_to((P, 8, T)),
                op=AL.mult)

            # ---- lhs_sT (16, 512) = sum_t1 (Ws2*w1e[t1]).T @ X^T[t1]
            lsTp = pss.tile([T, N], f32, tag="pssT", name="lst")
            for s in range(8):
                nc.tensor.matmul(lsTp[:], Ws2w[:, s, :], Xw[s][:, :],
                                 start=(s == 0), stop=(s == 7))
            lsT = sg.tile([T, N], bf16, tag="lsts")
            nc.scalar.copy(lsT[:], lsTp[:])

            # ---- rhs_s (16, 512) = Et-weighted rhs3
            rsp = pss.tile([T, N], f32, tag="pssT", name="rsp")
            nc.tensor.matmul(rsp[:], Et[:], R48[0:16, :], start=True, stop=True)
            rss = sg.tile([T, N], bf16, tag="rss")
            nc.scalar.copy(rss[:], rsp[:])

            # ---- P chunks + G = sigmoid(P + bs)
            for k in range(CH):
                Pp = psP.tile([P, N], f32, tag="pp", name="pp")
                nc.tensor.matmul(Pp[:], lsT[:, k * P:(k + 1) * P], rss[:],
                                 start=True, stop=True)
                nc.vector.tensor_tensor(G[k][:], Pp[:], bs[k][:], op=AL.add)
                nc.scalar.activation(G[k][:], G[k][:], AF.Sigmoid)

            # ---- M1T chunks (c-part, r) + masked softmax -> A1T, dS
            for c in range(CH):
                Mp = psM.tile([P, N], f32, tag="mp", name="mp")
                for k in range(CH):
                    nc.tensor.matmul(Mp[:], G[k][:, c * P:(c + 1) * P], VsT[k][:],
                                     start=(k == 0), stop=(k == CH - 1))
                sme = sg.tile([P, 1], f32, tag=f"sme{c}", name=f"sme{c}")
                nc.scalar.activation(Ex[c][:], Mp[:], AF.Exp,
                                     scale=1.0, accum_out=sme[:, 0:1])
                rcp = sg.tile([P, 1], f32, tag=f"rcp{c}", name=f"rcp{c}")
                nc.vector.reciprocal(rcp[:], sme[:])
                # A1T = (Ex * rcp) * WT   (= S^T ⊙ W^T)
                nc.vector.scalar_tensor_tensor(A1T[c][:], Ex[c][:], rcp[:, 0:1],
                                               WT[c][:], op0=AL.mult, op1=AL.mult)
                # diag: dS = sum_r (Ex*rcp)*I over the diagonal block
                dtmp = sg.tile([P, P], bf16, tag="dtmp")
                nc.vector.scalar_tensor_tensor(dtmp[:], Ex[c][:, c * P:(c + 1) * P],
                                               rcp[:, 0:1], I128b[:],
                                               op0=AL.mult, op1=AL.mult)
                nc.vector.tensor_reduce(dSv[c][:], dtmp[:], axis=AX.X, op=AL.add)

            # ---- dS row + broadcast tile (128, 512)
            dSrp = pss.tile([1, N], f32, tag="pssT", name="dsrp")
            for c in range(CH):
                nc.tensor.transpose(dSrp[:, c * P:(c + 1) * P], dSv[c][:], I128f[:])
            dSrow = sg.tile([1, N], bf16, tag="dsrow")
            nc.scalar.copy(dSrow[:], dSrp[:])
            dSBp = pss.tile([P, N], f32, tag="pssT", name="dsbp")
            nc.tensor.matmul(dSBp[:], ones1[:], dSrow[:], start=True, stop=True)
            nc.scalar.copy(dSB[:], dSBp[:])

            # ---- Tx0 in n-layout (all t at once)
            for k in range(CH):
                nc.vector.tensor_scalar(Tx0n[k][:], Xn[k][:], dSv[k][:, 0:1], None,
                                        op0=AL.mult)

        if stage <= 2:
            dbg = sg.tile([P, N], f32, tag="dbg")
            nc.vector.tensor_copy(dbg[:], G[0][:] if stage == 2 else A1T[0][:])
            nc.sync.dma_start(out=ZoutD[0:P, 0:N], in_=dbg[:])
        # =====================================================
        # Cheb + conv per pair
        # =====================================================
        if stage >= 3:
         with tc.tile_pool(name="psb", bufs=3, space="PSUM") as psb, \
             tc.tile_pool(name="pst", bufs=4, space="PSUM") as pst, \
             tc.tile_pool(name="xhp", bufs=7) as xhp, \
             tc.tile_pool(name="txp", bufs=3) as txp:

            XhP = {-1: zerot, 5: zerot}

            def conv_pair(q):
                # time conv + residual; Z = relu(... + bt + br)
                TD = psb.tile([P, N], f32, tag="big", name="big")
                nc.tensor.matmul(TD[:], Lprev[:], XhP[q - 1][:], start=True, stop=False)
                nc.tensor.matmul(TD[:], Lmid[:], XhP[q][:], start=False, stop=False)
                nc.tensor.matmul(TD[:], Lnext[:], XhP[q + 1][:], start=False, stop=False)
                nc.tensor.matmul(TD[:], WrP[:], Xw[q][:], start=False, stop=True)
                ZT = txp.tile([P, N], bf16, tag="zt", name="zt")
                nc.scalar.activation(ZT[:], TD[:], AF.Relu, bias=btr[:, 0:1], scale=1.0)
                # transpose to n-layout: Zbig[:, q*128 : (q+1)*128]
                for k in range(CH):
                    pt = pst.tile([P, P], bf16, tag="tr", name="tr")
                    nc.tensor.transpose(pt[:], ZT[:, k * P:(k + 1) * P], I128b[:])
                    if k % 2:
                        nc.vector.tensor_copy(Zbig[k][:, q * P:(q + 1) * P], pt[:])
                    else:
                        nc.scalar.copy(Zbig[k][:, q * P:(q + 1) * P], pt[:])

            for q in range(5):
                # Tx0 pair tile (v,f) x n
                Tx0P = txp.tile([P, N], bf16, tag="tx0p", name="tx0p")
                nc.gpsimd.tensor_tensor(Tx0P[:], Xw[q][:], dSB[:], op=AL.mult)

                # Tx1T pair = sum_k Tx0n-slice.T @ A1T
                TA = psb.tile([P, N], f32, tag="big", name="big")
                for k in range(CH):
                    lhs = Tx0n[k][:, 2 * q * F:(2 * q + 2) * F]
                    nc.tensor.matmul(TA[:], lhs, A1T[k][:],
                                     start=(k == 0), stop=(k == CH - 1))
                Tx1T = txp.tile([P, N], bf16, tag="tx1t", name="tx1t")
                nc.vector.tensor_copy(Tx1T[:], TA[:])

                # Tx1 back to n-layout (lhsT for next prop): all 4 transposes
                # land in one psum tile -> single copy
                ptA = pst.tile([P, N], bf16, tag="tr", name="tr")
                for k in range(CH):
                    nc.tensor.transpose(ptA[:, k * P:(k + 1) * P],
                                        Tx1T[:, k * P:(k + 1) * P], I128b[:])
                Tx1n = txp.tile([P, N], bf16, tag="tx1n", name="tx1n")
                if q % 2:
                    nc.vector.tensor_copy(Tx1n[:], ptA[:])
                else:
                    nc.scalar.copy(Tx1n[:], ptA[:])

                # Tx2T = 2 * (W @ Tx1)^T - Tx0T
                TB = psb.tile([P, N], f32, tag="big", name="big")
                for k in range(CH):
                    nc.tensor.matmul(TB[:], Tx1n[:, k * P:(k + 1) * P], WT[k][:],
                                     start=(k == 0), stop=(k == CH - 1))
                Tx2T = txp.tile([P, N], bf16, tag="tx2t", name="tx2t")
                nc.vector.scalar_tensor_tensor(Tx2T[:], TB[:], 2.0, Tx0P[:],
                                               op0=AL.mult, op1=AL.subtract)

                # out^T = sum_k WcP[k].T @ TxT ; Xhat = relu(out + b_cheb)
                TC = psb.tile([P, N], f32, tag="big", name="big")
                nc.tensor.matmul(TC[:], WcP[0][:], Tx0P[:], start=True, stop=False)
                nc.tensor.matmul(TC[:], WcP[1][:], Tx1T[:], start=False, stop=False)
                nc.tensor.matmul(TC[:], WcP[2][:], Tx2T[:], start=False, stop=True)
                Xh = xhp.tile([P, N], bf16, tag="xh", name="xh")
                nc.scalar.activation(Xh[:], TC[:], AF.Relu, bias=bch[:, 0:1], scale=1.0)
                XhP[q] = Xh

            for q in range(5):
                conv_pair(q)

        # =====================================================
        # LayerNorm over F per (n, slot) + store
        # =====================================================
        if stage >= 4:
         with tc.tile_pool(name="lnp", bufs=2) as lnp:
            NS2 = 5
            for hh in range(2):
              for c in range(CH):
                zpart = Zbig[c][:, hh * NS2 * F:(hh + 1) * NS2 * F]
                Z3 = zpart.rearrange("p (s f) -> p s f", f=F)
                sq = lnp.tile([P, NS2 * F], f32, tag="sq", name="sq")
                nc.gpsimd.tensor_tensor(sq[:], zpart, zpart, op=AL.mult)
                s1 = sg.tile([P, NS2], f32, tag=f"s1{c}{hh}", name=f"s1{c}{hh}")
                s2 = sg.tile([P, NS2], f32, tag=f"s2{c}{hh}", name=f"s2{c}{hh}")
                nc.vector.tensor_reduce(s1[:], Z3, axis=AX.X, op=AL.add)
                nc.vector.tensor_reduce(s2[:], sq[:].rearrange("p (s f) -> p s f", f=F),
                                        axis=AX.X, op=AL.add)
                mean = sg.tile([P, NS2], f32, tag=f"mn{c}{hh}", name=f"mn{c}{hh}")
                nc.vector.tensor_scalar(mean[:], s1[:], 1.0 / F, None, op0=AL.mult)
                var = sg.tile([P, NS2], f32, tag=f"vr{c}{hh}", name=f"vr{c}{hh}")
                nc.vector.tensor_tensor(var[:], mean[:], mean[:], op=AL.mult)
                nc.vector.scalar_tensor_tensor(var[:], s2[:], 1.0 / F, var[:],
                                               op0=AL.mult, op1=AL.subtract)
                sd = sg.tile([P, NS2], f32, tag=f"sd{c}{hh}", name=f"sd{c}{hh}")
                nc.scalar.activation(sd[:], var[:], AF.Sqrt, bias=epsP[:, 0:1], scale=1.0)
                rstd = sg.tile([P, NS2], f32, tag=f"rs{c}{hh}", name=f"rs{c}{hh}")
                nc.vector.reciprocal(rstd[:], sd[:])
                nmr = sg.tile([P, NS2], f32, tag=f"nm{c}{hh}", name=f"nm{c}{hh}")
                nc.vector.tensor_tensor(nmr[:], mean[:], rstd[:], op=AL.mult)
                nc.vector.tensor_scalar(nmr[:], nmr[:], -1.0, None, op0=AL.mult)

                eng = nc.vector if c < 3 else nc.gpsimd
                rB = rstd[:].unsqueeze(2).broadcast_to((P, NS2, F))
                nB = nmr[:].unsqueeze(2).broadcast_to((P, NS2, F))
                gB = gamB[:].unsqueeze(1).broadcast_to((P, NS2, F))
                bB = betB[:].unsqueeze(1).broadcast_to((P, NS2, F))
                scr = lnp.tile([P, NS2 * F], f32, tag="scr", name="scr")
                S3 = scr[:].rearrange("p (s f) -> p s f", f=F)
                O3 = lnout[c][:, hh * NS2 * F:(hh + 1) * NS2 * F].rearrange(
                    "p (s f) -> p s f", f=F)
                eng.tensor_tensor(S3, Z3, rB, op=AL.mult)
                eng.tensor_tensor(S3, S3, nB, op=AL.add)
                eng.tensor_tensor(S3, S3, gB, op=AL.mult)
                eng.tensor_tensor(O3, S3, bB, op=AL.add)
                nc.sync.dma_start(
                    out=ZoutD[c * P:(c + 1) * P, hh * NS2 * F:(hh + 1) * NS2 * F],
                    in_=lnout[c][:, hh * NS2 * F:(hh + 1) * NS2 * F])

    nc.compile()
    return nc


def _host_prep(inputs):
    import ml_dtypes
    bf = ml_dtypes.bfloat16

    X = np.asarray(inputs['X'], np.float32)
    edge_index = np.asarray(inputs['edge_index'])
    U1 = np.asarray(inputs['U1'], np.float32)
    U2 = np.asarray(inputs['U2'], np.float32)
    U3 = np.asarray(inputs['U3'], np.float32)
    be = np.asarray(inputs['be'], np.float32)
    Ve = np.asarray(inputs['Ve'], np.float32)
    Ws1 = np.asarray(inputs['Ws1'], np.float32)
    Ws2 = np.asarray(inputs['Ws2'], np.float32)
    Ws3 = np.asarray(inputs['Ws3'], np.float32)
    bs = np.asarray(inputs['bs'], np.float32)
    Vs = np.asarray(inputs['Vs'], np.float32)
    W_cheb = np.asarray(inputs['W_cheb'], np.float32)
    b_cheb = np.asarray(inputs['b_cheb'], np.float32)
    Wt = np.asarray(inputs['Wt'], np.float32)
    bt = np.asarray(inputs['bt'], np.float32)
    Wr = np.asarray(inputs['Wr'], np.float32)
    br = np.asarray(inputs['br'], np.float32)
    gamma = np.asarray(inputs['gamma'], np.float32)
    beta = np.asarray(inputs['beta'], np.float32)

    # dense symmetric-norm matrix (self-loop +I/-I terms cancel)
    row, col = edge_index[0].astype(np.int64), edge_index[1].astype(np.int64)
    deg = np.zeros(N, np.float32)
    np.add.at(deg, row, 1.0)
    dis = np.where(deg > 0, 1.0 / np.sqrt(np.maximum(deg, 1.0)), 0.0).astype(np.float32)
    wn = -dis[row] * dis[col]
    W = np.zeros((N, N), np.float32)
    np.add.at(W, (row, col), wn)

    # conv block matrices: L[(v,fi),(u,fo)] = Wt[fo,fi,0,dt]
    WtT = [np.ascontiguousarray(Wt[:, :, 0, d].T) for d in range(3)]  # (fi,fo)
    Z64 = np.zeros((F, F), np.float32)
    Lmid = np.block([[WtT[1], WtT[0]], [WtT[2], WtT[1]]]).astype(bf)
    Lprev = np.block([[Z64, Z64], [WtT[0], Z64]]).astype(bf)
    Lnext = np.block([[Z64, WtT[2]], [Z64, Z64]]).astype(bf)
    WrT = np.ascontiguousarray(Wr[:, :, 0, 0].T)
    WrP = np.block([[WrT, Z64], [Z64, WrT]]).astype(bf)
    WcP = np.stack([np.block([[W_cheb[k], Z64], [Z64, W_cheb[k]]]) for k in range(3)]
                   ).astype(bf)

    Wpk = np.stack([WcP[0], WcP[1], WcP[2], Lprev, Lmid, Lnext, WrP])

    Pf = np.zeros((P, 274), np.float32)
    Pf[:, 0:64] = gamma[None, :]
    Pf[:, 64:128] = beta[None, :]
    Pf[:, 128] = np.tile(b_cheb, 2)
    Pf[:, 129] = np.tile(bt + br, 2)
    Pf[:, 130:258] = np.eye(P, dtype=np.float32)

    shared = {
        'bs': bs[0].astype(bf),
        'VsT': np.ascontiguousarray(Vs.T).astype(bf),
        'WT': np.ascontiguousarray(W.T).astype(bf),
        'Wpk': Wpk,
    }

    in_maps = []
    for core in range(8):
        b, h = core // 2, core % 2
        tmap = list(range(16)) if h == 0 else list(range(6, 16)) + list(range(6))
        Xp = X[b][:, :, tmap]                              # (N, F, 16)
        Xn = np.ascontiguousarray(Xp.transpose(0, 2, 1).reshape(N, T * F)).astype(bf)
        Xw = np.ascontiguousarray(Xp.transpose(2, 1, 0).reshape(8, P, N)).astype(bf)
        UW = np.zeros((8, P, 48), np.float32)
        for tp in range(16):
            s, v = tp // 2, tp % 2
            UW[s, 64 * v:64 * v + 64, tp] = Ws3
            UW[s, 64 * v:64 * v + 64, 32 + tp] = U3
        Pb = np.zeros((P, 1061), np.float32)
        Pb[:, 0:4] = U1.reshape(4, P).T
        Pb[:, 4:20] = np.vstack([Ws2, Ws2])
        Pb[0:16, 20:36] = Ve[np.ix_(tmap, tmap)].T
        Pb[0:16, 36] = Ws1[tmap]
        Pb[0, 37:165] = 1.0
        Pb[:, 165:293] = np.eye(P, dtype=np.float32)
        Pb[0:64, 293:805] = U2
        Pb[0, 805:1061] = np.eye(T, dtype=np.float32).reshape(-1)
        Pfc = Pf.copy()
        Pfc[0:16, 258:274] = be[0][np.ix_(tmap, tmap)]
        m = dict(shared)
        m.update({
            'Xn': Xn, 'Xw': Xw, 'UW': UW.astype(bf),
            'Pb': Pb.astype(bf), 'Pf': Pfc,
        })
        in_maps.append(m)
    return in_maps


def kernel(**inputs):
    import sys
    if '/opt/trn_rl_repo' not in sys.path:
        sys.path.insert(0, '/opt/trn_rl_repo')
    from concourse.bass_utils import run_bass_kernel_spmd

    if 'nc' not in _CACHE:
        _CACHE['nc'] = _build_program()
    nc = _CACHE['nc']

    in_maps = _host_prep(inputs)
    res = run_bass_kernel_spmd(nc, in_maps, list(range(8)))
    out = np.zeros((B, N, F, T), np.float32)
    for core in range(8):
        b, h = core // 2, core % 2
        Z = np.asarray(res.results[core]['Zout']).reshape(N, NSLOT, F)
        wstart = 0 if h == 0 else 6
        jlo = 0 if h == 0 else 2
        for j in range(jlo, jlo + 8):
            out[b, :, :, wstart + j] = Z[:, j, :]
    return out

